# revision 35
# baseline (speedup 1.0000x reference)
"""Trainium2 Bass kernel for nn_EnsembleNet3 (gnn_message_passing).

Self-contained: takes full inputs (as produced by setup_inputs()), shards the
B=32 graph dim over 8 NeuronCores (4 graphs/core), runs the graph stack fully
on-device, and computes the [B,1536] head sharded over output columns with
AllGathers between layers.

v2: stage-major emission (software pipelining across the 4 graphs per core),
EdgeConv-1 overlapped with the kNN-threshold selection, transposed adjacency
masks generated directly by an augmented matmul (no PE mask transposes),
statistical secant init (7 iters instead of 9), per-rank strided gathers,
shared -|y|^2/2 row matmul for the EdgeConv distance (half-scale Q2 keeps the
ranking), GPSIMD offload of the EdgeConv adds, DVE-based batchnorm stats, and
a reoriented [32,192] head with layer weights prefetched one ahead.
"""
import os
from contextlib import ExitStack

import numpy as np
import ml_dtypes

import concourse.bass as bass
import concourse.bacc as bacc
import concourse.tile as tile
from concourse import mybir
from concourse._compat import with_exitstack

F32 = mybir.dt.float32
F16 = mybir.dt.float16
BF16 = mybir.dt.bfloat16
U16 = mybir.dt.uint16
I16 = mybir.dt.int16
U32 = mybir.dt.uint32
U8 = mybir.dt.uint8
ALU = mybir.AluOpType
ACTF = mybir.ActivationFunctionType
AXX = mybir.AxisListType.X

B, N, F, W = 32, 512, 6, 128
NT = N // 128
GPC = 4
NCORES = 8
K101 = 101
SEL_ITERS = 6
SEL_TARGET = float(K101 + 4)
Z0_QUANT = 0.8236186950515331  # ndtri(1 - 105/512)
U_LO, U_HI = -64.0, 64.0
MASK_DELTA = 1e-3  # tolerance so the recomputed Q keeps the rank-101 element
DIM2 = 1536
HSH = DIM2 // NCORES
LIN_D = 5
DVE_COLS = 8     # selection count passes: cols < DVE_COLS on DVE, rest on ACT


def _fold_tag(Wk, b):
    W0, W1, W2 = Wk[0], Wk[1], Wk[2]
    c1, c2 = 1.0 / 100.0, 1.0 / 10000.0
    return (
        (W0 - W1 * c1 + W2 * c2).astype(np.float32),
        (W1 * c1 - 2.0 * W2 * c2).astype(np.float32),
        (W2 * c2).astype(np.float32),
        b.astype(np.float32),
    )


def prep_host(inputs, core):
    inp = {k: np.asarray(v) for k, v in inputs.items()}
    x = inp['x'].astype(np.float32).reshape(B, N, F)
    xs = x[GPC * core:GPC * (core + 1)]
    f32 = np.float32

    # --- per-graph augmented input [8, 1024]: T-block | R-block.
    # T (cols 0:N):   rows 0:6 xT, row 6 ones   (Q lhsT = rows 0:7 chunk)
    # R (cols N:2N):  rows 0:6 2xT, row 6 -|x|^2, row 7 ones
    #   (Q rhs = rows 0:7 of R; QT' lhsT = rows 0:8 chunk of R) ---
    xt = xs.transpose(0, 2, 1)                       # [GPC, 6, 512]
    ssq = np.sum(xt * xt, axis=1)                    # [GPC, 512]
    xaug = np.zeros((GPC, 8, 2 * N), f32)
    xaug[:, 0:F, 0:N] = xt
    xaug[:, F, 0:N] = 1.0
    xaug[:, 0:F, N:2 * N] = 2.0 * xt
    xaug[:, F, N:2 * N] = -ssq
    xaug[:, F + 1, N:2 * N] = 1.0

    # --- node-major x in bf16 for TAG layer-0 aggregation: [128, 24] ---
    xnmh = np.zeros((GPC, 128, NT * F), np.float32)
    for t in range(NT):
        xnmh[:, :, F * t:F * (t + 1)] = xs[:, 128 * t:128 * (t + 1), :]
    xnmh = xnmh.astype(ml_dtypes.bfloat16)

    # --- const pack [128, cols] ---
    cols = {}
    blobs = []
    off = 0

    def put(name, arr2d):
        nonlocal off
        a = np.asarray(arr2d, f32)
        pad = np.zeros((128, a.shape[1]), f32)
        pad[:a.shape[0]] = a
        cols[name] = (off, a.shape[1])
        blobs.append(pad)
        off += a.shape[1]

    put('eye', np.eye(128, dtype=f32))
    put('iota8', np.broadcast_to(np.arange(8, dtype=f32), (128, 8)))
    put('iotaj', np.arange(512, dtype=f32).reshape(4, 128).T)
    for li, (Wk, bk) in enumerate(
            [(inp['tag1_W'], inp['tag1_b']),
             (inp['tag_W'][0], inp['tag_b'][0]),
             (inp['tag_W'][1], inp['tag_b'][1])]):
        w0, w1, w2, bb = _fold_tag(Wk, bk)
        put(f'tagw{li}', np.concatenate([w0, w1, w2], axis=1))
        put(f'tagb{li}', bb.reshape(128, 1))
    W1 = inp['p1_W1'].astype(f32)
    put('ec1_a', W1[:F] - W1[F:])
    put('ec1_g', W1[F:])
    put('ec1_b1', inp['p1_b1'].astype(f32).reshape(128, 1))
    put('ec1_w2', inp['p1_W2'].astype(f32))
    put('ec1_b2', inp['p1_b2'].astype(f32).reshape(128, 1))
    for f in range(2):
        Wf = inp['pf_W'][f].astype(f32)
        put(f'ec{f+2}_a', Wf[:W] - Wf[W:])
        put(f'ec{f+2}_g', Wf[W:])
        put(f'ec{f+2}_b', inp['pf_b'][f].astype(f32).reshape(128, 1))
    put('bn_scale', inp['bn_gamma'].astype(f32).reshape(12, 128).T)
    put('bn_shift', inp['bn_beta'].astype(f32).reshape(12, 128).T)
    put('outW', inp['out_W'].astype(f32).reshape(12, 128).T)
    put('outb', np.full((1, 1), float(inp['out_b'][0]), f32))
    sl = slice(HSH * core, HSH * (core + 1))
    put('linB', inp['lin_b'].astype(f32)[:, sl].reshape(1, LIN_D * HSH))
    wpack = np.concatenate(blobs, axis=1)
    assert wpack.shape[1] == WPACK_COLS, (wpack.shape, off)
    assert all(cols[k] == WOFF[k] for k in cols), "WOFF mismatch"

    linW = inp['lin_W'].astype(f32)
    d = {
        'xaug': np.ascontiguousarray(xaug),
        'xnmh': np.ascontiguousarray(xnmh),
        'wpack': np.ascontiguousarray(wpack),
        'linW': np.ascontiguousarray(
            linW[:, :, sl].reshape(LIN_D, 12, 128, HSH).astype(np.float16)),
    }
    return d


def _woff_table():
    off = 0
    tab = {}
    def put(name, w):
        nonlocal off
        tab[name] = (off, w)
        off += w
    put('eye', 128); put('iota8', 8); put('iotaj', 4)
    for li in range(3):
        put(f'tagw{li}', 384); put(f'tagb{li}', 1)
    put('ec1_a', 128); put('ec1_g', 128); put('ec1_b1', 1)
    put('ec1_w2', 128); put('ec1_b2', 1)
    for f in range(2):
        put(f'ec{f+2}_a', 128); put(f'ec{f+2}_g', 128); put(f'ec{f+2}_b', 1)
    put('bn_scale', 12); put('bn_shift', 12)
    put('outW', 12); put('outb', 1)
    put('linB', LIN_D * HSH)
    return tab, off


WOFF, WPACK_COLS = _woff_table()


@with_exitstack
def core_program(ctx: ExitStack, tc: tile.TileContext, io: dict):
    nc = tc.nc
    P = 128

    const = ctx.enter_context(tc.tile_pool(name="const", bufs=1))
    pq = ctx.enter_context(tc.tile_pool(name="pq", bufs=16))
    pmask = ctx.enter_context(tc.tile_pool(name="pmask", bufs=16))
    ppay = ctx.enter_context(tc.tile_pool(name="ppay", bufs=4))
    pnm = ctx.enter_context(tc.tile_pool(name="pnm", bufs=4))
    pP = ctx.enter_context(tc.tile_pool(name="pP", bufs=8))
    pmx = ctx.enter_context(tc.tile_pool(name="pmx", bufs=2))
    pyT = ctx.enter_context(tc.tile_pool(name="pyT", bufs=4))
    phT = ctx.enter_context(tc.tile_pool(name="phT", bufs=4))
    pu = ctx.enter_context(tc.tile_pool(name="pu", bufs=4))
    pub = ctx.enter_context(tc.tile_pool(name="pub", bufs=4))
    pwork = ctx.enter_context(tc.tile_pool(name="pwork", bufs=2))
    pmar = ctx.enter_context(tc.tile_pool(name="pmar", bufs=2))
    pst = ctx.enter_context(tc.tile_pool(name="pst", bufs=1))
    phw = ctx.enter_context(tc.tile_pool(name="phw", bufs=2))
    psb = ctx.enter_context(tc.tile_pool(name="psb", bufs=6, space="PSUM"))
    pss = ctx.enter_context(tc.tile_pool(name="pss", bufs=2, space="PSUM"))

    def bank(pp=P, nn=N):
        t5 = psb.tile([P, N], F32, tag="bank", name="bank")
        return t5[0:pp, 0:nn]

    def bank16(pp=P, nn=N):
        # full-bank padded F16 PSUM view (PE-write/engine-read same-bank rule)
        t5 = psb.tile([P, 2 * N], F16, tag="bank", name="bank16")
        return t5[0:pp, 0:nn]

    def sbank(pp=P, nn=N):
        t5 = pss.tile([P, N], F32, tag="sbank", name="sbank")
        return t5[0:pp, 0:nn]

    def dma(dst, src):
        nc.sync.dma_start(dst, src)

    # ---- constants: one packed DMA ----
    wp = const.tile([P, WPACK_COLS], F32, tag="wpack", name="wpack")
    dma(wp[:], io['wpack'][:])

    def wslice(name, rows=128):
        o, w = WOFF[name]
        return wp[0:rows, o:o + w]

    eye = wslice('eye')
    iota8 = wslice('iota8')
    iotaj = wslice('iotaj')
    ones_f16 = const.tile([1, P], F16, padded_shape=[128, P])
    nc.any.memset(ones_f16[:], 1.0)
    eye16 = const.tile([P, P], F16)
    nc.vector.tensor_copy(eye16[:], wp[0:128, 0:128])
    ones32 = const.tile([1, 32], F32, padded_shape=[128, 32])
    nc.any.memset(ones32[:], 1.0)
    allones = const.tile([P, P], F32)
    nc.any.memset(allones[:], 1.0)
    eps_col = const.tile([P, 1], F32)
    nc.any.memset(eps_col[:], 1e-9)
    del_col = const.tile([P, 1], F32)
    nc.any.memset(del_col[:], MASK_DELTA)

    tagw, tagb = [], []
    for li in range(3):
        fin = F if li == 0 else W
        wt = wslice(f'tagw{li}', fin)
        w16 = const.tile([fin, 384], F16, padded_shape=[128, 384],
                         name=f"tagw16_{li}")
        nc.vector.tensor_copy(w16[:], wt)
        tagw.append([w16[:, 128 * k:128 * (k + 1)] for k in range(3)])
        tagb.append(wslice(f'tagb{li}'))

    ec1_a = wslice('ec1_a', F)
    ec1_g = wslice('ec1_g', F)
    ec1_w2 = wslice('ec1_w2')
    ec1_b1 = wslice('ec1_b1')
    ec1_b2 = wslice('ec1_b2')
    ecf_a = [wslice('ec2_a'), wslice('ec3_a')]
    ecf_g = [wslice('ec2_g'), wslice('ec3_g')]
    ecf_b = [wslice('ec2_b'), wslice('ec3_b')]

    # ---- head weight stream: two slots, prefetched one layer ahead ----
    wtl = [None] * LIN_D

    def load_linw(li):
        wt = phw.tile([P, 12 * HSH], F16, tag="linW", name="linW")
        dma(wt[:].rearrange("a (k b) -> a k b", k=12),
            io['linW'][li].rearrange("k a b -> a k b"))
        wtl[li] = wt

    load_linw(0)
    load_linw(1)

    # ---- inputs per graph ----
    xaug, xnmh = [], []
    for g in range(GPC):
        xa = pst.tile([8, 2 * N], F32, tag=f"xaug{g}", name=f"xaug{g}")
        dma(xa[:], io['xaug'][g])
        xaug.append(xa)
        xh = pst.tile([P, NT * F], BF16, tag=f"xnmh{g}", name=f"xnmh{g}")
        dma(xh[:], io['xnmh'][g])
        xnmh.append(xh)

    def xT(g):   # [6, 512] f32 feature-major x
        return xaug[g][0:F, 0:N]

    xT16l = []
    for g in range(GPC):
        x16 = pst.tile([F, N], F16, tag=f"xT16{g}", name=f"xT16{g}",
                       padded_shape=[128, N])
        nc.vector.tensor_copy(x16[:], xaug[g][0:F, 0:N])
        xT16l.append(x16)

    # ---- selection state ----
    NC16 = GPC * NT
    ssum = pst.tile([P, NC16], F32, tag="ssum", name="ssum")
    qq = pst.tile([P, NC16], F32, tag="qq", name="qq")
    st_u = pst.tile([P, NC16], F32, tag="st_u", name="st_u")
    st_ul = pst.tile([P, NC16], F32, tag="st_ul", name="st_ul")
    st_uh = pst.tile([P, NC16], F32, tag="st_uh", name="st_uh")
    st_cl = pst.tile([P, NC16], F32, tag="st_cl", name="st_cl")
    st_ch = pst.tile([P, NC16], F32, tag="st_ch", name="st_ch")
    cnt = pst.tile([P, NC16], F32, tag="cnt", name="cnt")
    tmp_a = pst.tile([P, NC16], F32, tag="tmp_a", name="tmp_a")
    tmp_b = pst.tile([P, NC16], F32, tag="tmp_b", name="tmp_b")
    tmp_m = pst.tile([P, NC16], U8, tag="tmp_m", name="tmp_m")
    junk_d = pst.tile([P, N], F32, tag="junk_d", name="junk_d")
    junk_a = pst.tile([P, N], F32, tag="junk_a", name="junk_a")
    nc.any.memset(st_ul[:], U_HI)
    nc.any.memset(st_cl[:], 0.0)
    nc.any.memset(st_uh[:], U_LO)
    nc.any.memset(st_ch[:], float(N))

    # ---- Q[i,j] = 2<x_i,x_j> - |x_j|^2 via K=7 augmented matmul ----
    Q = [[None] * NT for _ in range(GPC)]
    for g in range(GPC):
        for t in range(NT):
            col = 4 * g + t
            gps = bank()
            nc.tensor.matmul(gps, xaug[g][0:7, 128 * t:128 * (t + 1)],
                             xaug[g][0:7, N:2 * N], start=True, stop=True)
            qt = pq.tile([P, N], F32, tag="Q", name="Q")
            # copy with free-dim row-sum accumulation (for the stat init)
            nc.scalar.activation(qt[:], gps, ACTF.Identity,
                                 accum_out=ssum[:, col:col + 1])
            Q[g][t] = qt

    # row sum of squares for the stat init (DVE)
    for g in range(GPC):
        for t in range(NT):
            col = 4 * g + t
            nc.vector.scalar_tensor_tensor(
                junk_d[:], Q[g][t][:], 1.0, Q[g][t][:],
                op0=ALU.mult, op1=ALU.mult, accum_out=qq[:, col:col + 1])

    # ---- EC1 payloads (PE; independent of selection) ----
    # a1 feature-major fp32; g1 node-major fp16 (gather-matmul lhsT)
    a1l, g1nm = [], []
    for g in range(GPC):
        a1_ps = bank()
        nc.tensor.matmul(a1_ps, ec1_a, xT(g), start=True, stop=True)
        a1 = ppay.tile([P, N], F32, tag="pay", name="a1")
        nc.scalar.activation(a1[:], a1_ps, ACTF.Identity, bias=ec1_b1)
        gnm_ps = bank()
        for jc in range(NT):
            nc.tensor.matmul(gnm_ps[:, 128 * jc:128 * (jc + 1)],
                             xaug[g][0:F, 128 * jc:128 * (jc + 1)],
                             ec1_g, start=True, stop=True)
        gnm = pnm.tile([P, N], F32, tag="nm", name="g1nm")
        nc.scalar.activation(gnm[:], gnm_ps, ACTF.Identity)
        a1l.append(a1); g1nm.append(gnm)

    # ---- one-hot gather helpers (PE-based; no GPSIMD) ----
    def marshal_i12(Qt):
        """top-3 idx of each Q row -> i12 [128, 12] f32 (col 3t+l)."""
        i12 = pmar.tile([P, 12], F32, tag="i12", name="i12")
        for t in range(NT):
            m8 = pmar.tile([P, 8], F32, tag="m8g", name="m8g")
            nc.vector.max(m8[:], Qt[t][:])
            i8 = pmar.tile([P, 8], U32, tag="i8g", name="i8g")
            nc.vector.max_index(i8[:], m8[:], Qt[t][:])
            nc.vector.tensor_copy(i12[:, 3 * t:3 * t + 3], i8[:, 1:4])
        return i12

    def onehot_rows(i12):
        """i12 -> per-rank one-hot matrices P_l [j, i] (list of 3 x 4 chunks).

        P_l[j, i] = 1 iff node j is the rank-(l+1) neighbour of node i.
        Built by broadcasting the index row and comparing against iota(j).
        """
        t1_ps = bank(12, P)
        nc.tensor.transpose(t1_ps, i12[:], eye)
        tsg = pmar.tile([12, P], F32, tag="tsg", name="tsg",
                        padded_shape=[128, P])
        nc.scalar.activation(tsg[:], t1_ps, ACTF.Identity)
        Pl = []
        for l in range(3):
            row_ps = bank(1, N)
            for t in range(NT):
                nc.tensor.matmul(row_ps[0:1, 128 * t:128 * (t + 1)],
                                 eye[0:12, 3 * t + l:3 * t + l + 1],
                                 tsg[0:12, :], start=True, stop=True)
            irow = pmar.tile([1, N], F16, tag="irow", name="irow",
                             padded_shape=[128, N])
            nc.scalar.activation(irow[:], row_ps, ACTF.Identity)
            ib_ps = bank()
            nc.tensor.matmul(ib_ps, ones_f16[0:1, :], irow[:],
                             start=True, stop=True)
            chunks = []
            for jc in range(NT):
                pc = pP.tile([P, N], F32, tag="P", name="Pl")
                nc.vector.tensor_scalar(pc[:], ib_ps, iotaj[:, jc:jc + 1],
                                        0.0, op0=ALU.is_equal)
                chunks.append(pc)
            Pl.append(chunks)
        return Pl

    def gather_mm(pay_nm, chunks):
        """ga[feat, i] = payload[feat, idx(i)] via one-hot matmul (PSUM)."""
        ga_ps = bank()
        for jc in range(NT):
            nc.tensor.matmul(ga_ps, pay_nm[:, 128 * jc:128 * (jc + 1)],
                             chunks[jc][:], start=(jc == 0),
                             stop=(jc == NT - 1))
        return ga_ps

    zpack = [pst.tile([P, 12], F32, tag=f"zpack{g}", name=f"zpack{g}")
             for g in range(GPC)]

    # ---- EC1: one-hot gather + MLP; runs concurrently with selection ----
    mxl = []
    for g in range(GPC):
        Pl = onehot_rows(marshal_i12(Q[g]))
        mx = pmx.tile([P, N], F32, tag="mx", name="mx")
        for l in range(3):
            ga_ps = gather_mm(g1nm[g][:], Pl[l])
            hid = pnm.tile([P, N], F32, tag="hid", name="hid")
            nc.vector.tensor_tensor(hid[:], a1l[g][:], ga_ps, op=ALU.add)
            nc.scalar.activation(hid[:], hid[:], ACTF.Lrelu, alpha=0.01)
            m_ps = bank()
            nc.tensor.matmul(m_ps, ec1_w2, hid[:], start=True, stop=True)
            if l == 0:
                nc.vector.tensor_copy(mx[:], m_ps)
            else:
                nc.vector.tensor_tensor(mx[:], mx[:], m_ps, op=ALU.max)
        mxl.append(mx)

    yTl = [None] * GPC
    for g in range(GPC):
        yT = pyT.tile([P, N], F32, tag="yT", name="yT")
        nc.scalar.activation(yT[:], mxl[g][:], ACTF.Lrelu, bias=ec1_b2,
                             alpha=0.01, accum_out=zpack[g][:, 6:7])
        nc.vector.tensor_reduce(zpack[g][:, 9:10], yT[:], axis=AXX, op=ALU.max)
        yTl[g] = yT

    # ---- statistical init for the count-secant ----
    mu_t = pst.tile([P, NC16], F32, tag="mu_t", name="mu_t")
    nc.vector.tensor_scalar(mu_t[:], ssum[:], 1.0 / N, 0.0, op0=ALU.mult)
    m2_t = pst.tile([P, NC16], F32, tag="m2_t", name="m2_t")
    nc.vector.tensor_tensor(m2_t[:], mu_t[:], mu_t[:], op=ALU.mult)
    var_t = pst.tile([P, NC16], F32, tag="var_t", name="var_t")
    nc.vector.scalar_tensor_tensor(var_t[:], qq[:], 1.0 / N, m2_t[:],
                                   op0=ALU.mult, op1=ALU.subtract)
    sd_t = pst.tile([P, NC16], F32, tag="sd_t", name="sd_t")
    nc.scalar.activation(sd_t[:], var_t[:], ACTF.Sqrt, bias=eps_col[:])
    nc.vector.scalar_tensor_tensor(st_u[:], sd_t[:], Z0_QUANT, mu_t[:],
                                   op0=ALU.mult, op1=ALU.add)

    # ---- lockstep count-secant selection ----
    for it in range(SEL_ITERS):
        for g in range(GPC):
            for t in range(NT):
                col = 4 * g + t
                ucol = st_u[:, col:col + 1]
                ccol = cnt[:, col:col + 1]
                if col < DVE_COLS:
                    nc.vector.tensor_scalar(
                        junk_d[:], Q[g][t][:], ucol, 0.0,
                        op0=ALU.is_ge, op1=ALU.add, accum_out=ccol)
                else:
                    nc.scalar.activation(
                        junk_a[:], Q[g][t][:], ACTF.Sign,
                        bias=ucol, scale=-1.0, accum_out=ccol)
        # ACT cols: c = 256 - s/2
        nc.vector.tensor_scalar(
            cnt[:, DVE_COLS:NC16], cnt[:, DVE_COLS:NC16], -0.5, 256.0,
            op0=ALU.mult, op1=ALU.add)
        nc.vector.tensor_scalar(
            tmp_m[:], cnt[:], float(K101) - 0.5, 0.0, op0=ALU.is_ge)
        nc.vector.copy_predicated(st_uh[:], tmp_m[:], st_u[:])
        nc.vector.copy_predicated(st_ch[:], tmp_m[:], cnt[:])
        nc.vector.tensor_scalar(
            tmp_m[:], cnt[:], float(K101) - 0.5, 0.0, op0=ALU.is_lt)
        nc.vector.copy_predicated(st_ul[:], tmp_m[:], st_u[:])
        nc.vector.copy_predicated(st_cl[:], tmp_m[:], cnt[:])
        if it == SEL_ITERS - 1:
            break
        nc.vector.tensor_tensor(tmp_a[:], st_ch[:], st_cl[:], op=ALU.subtract)
        nc.vector.reciprocal(tmp_a[:], tmp_a[:])
        nc.vector.scalar_tensor_tensor(
            tmp_b[:], st_ch[:], -SEL_TARGET, tmp_a[:], op0=ALU.add, op1=ALU.mult)
        nc.vector.tensor_scalar(
            tmp_b[:], tmp_b[:], 0.05, 0.95, op0=ALU.max, op1=ALU.min)
        nc.vector.tensor_tensor(tmp_a[:], st_ul[:], st_uh[:], op=ALU.subtract)
        nc.vector.tensor_tensor(tmp_a[:], tmp_a[:], tmp_b[:], op=ALU.mult)
        nc.vector.tensor_tensor(st_u[:], st_uh[:], tmp_a[:], op=ALU.add)

    # ---- endgame: exact 101st-largest of each Q row ----
    ustar = pst.tile([P, NC16], F32, tag="ustar", name="ustar")
    pos = pst.tile([P, NC16], F32, tag="pos", name="pos")
    nc.vector.tensor_scalar(pos[:], st_ch[:], -float(K101), 0.0, op0=ALU.add)
    nc.vector.tensor_scalar(pos[:], pos[:], 0.0, 7.0, op0=ALU.max, op1=ALU.min)
    for g in range(GPC):
        for t in range(NT):
            col = 4 * g + t
            zm = pwork.tile([P, N], F32, tag="zm", name="zm", bufs=1)
            nc.vector.tensor_scalar(
                zm[:], Q[g][t][:], st_uh[:, col:col + 1], -1e30,
                op0=ALU.is_lt, op1=ALU.mult)
            nc.vector.tensor_tensor(zm[:], zm[:], Q[g][t][:], op=ALU.subtract)
            m8 = pwork.tile([P, 8], F32, tag="m8e", name="m8e")
            nc.vector.max(m8[:], zm[:])
            msk8 = pwork.tile([P, 8], F32, tag="msk8", name="msk8")
            nc.vector.tensor_tensor(
                msk8[:], iota8,
                pos[:, col:col + 1].broadcast_to([P, 8]), op=ALU.is_equal)
            j8 = pwork.tile([P, 8], F32, tag="j8", name="j8")
            nc.vector.scalar_tensor_tensor(
                j8[:], m8[:], -1.0, msk8[:], op0=ALU.mult, op1=ALU.mult,
                accum_out=ustar[:, col:col + 1])

    # ---- -ustar+delta as a row per graph (for the transposed-mask mm) ----
    t1_ps = bank(16, P)
    nc.tensor.transpose(t1_ps, ustar[:], eye)
    t1sb = pst.tile([16, P], F32, tag="t1sb", name="t1sb", padded_shape=[128, P])
    nc.scalar.activation(t1sb[:], t1_ps, ACTF.Identity)
    urow = []
    for g in range(GPC):
        ur = pst.tile([1, N], F32, tag=f"urow{g}", name=f"urow{g}",
                      padded_shape=[128, N])
        row_ps = bank(1, N)
        for t in range(NT):
            nc.tensor.matmul(row_ps[0:1, 128 * t:128 * (t + 1)],
                             eye[0:16, 4 * g + t:4 * g + t + 1],
                             t1sb[0:16, :], start=True, stop=True)
        nc.scalar.activation(ur[:], row_ps, ACTF.Copy, scale=-1.0,
                             bias=MASK_DELTA)
        urow.append(ur)

    # ---- transposed masks directly: QT'[j,i] = Q[i,j] - ustar[i] + delta
    # matmul 1 (K=7): [2xT; -|x|^2]_chunk^T @ [xT; 1]
    # matmul 2 (K=1): ones^T @ (-ustar+delta) row ----
    maskT = [[None] * NT for _ in range(GPC)]
    for g in range(GPC):
        for jc in range(NT):
            qt_ps = bank()
            nc.tensor.matmul(qt_ps,
                             xaug[g][0:7, N + 128 * jc:N + 128 * (jc + 1)],
                             xaug[g][0:7, 0:N], start=True, stop=False)
            nc.tensor.matmul(qt_ps, allones[0:1, 0:128], urow[g][:],
                             start=False, stop=True)
            mT = pmask.tile([P, N], BF16, tag="m", name="maskT")
            nc.vector.tensor_scalar(mT[:], qt_ps, 0.0, 0.0, op0=ALU.is_ge)
            maskT[g][jc] = mT

    # ---- TAG layers interleaved with EC2/EC3 ----
    hTl = [xT16l[g][:] for g in range(GPC)]
    hnmBl = [[xnmh[g][:, F * t:F * (t + 1)] for t in range(NT)]
             for g in range(GPC)]

    def tag_layer(li):
        fin = F if li == 0 else W
        u1Tl, u1nBl, u2Tl = [], [], []
        for g in range(GPC):
            u1_ps = bank(fin, N)
            for jc in range(NT):
                nc.tensor.matmul(u1_ps, hnmBl[g][jc], maskT[g][jc][:],
                                 start=(jc == 0), stop=(jc == NT - 1))
            u1T = pu.tile([fin, N], F16, tag="uT", name="u1T",
                          padded_shape=[128, N])
            nc.scalar.activation(u1T[:], u1_ps, ACTF.Identity)
            u1Tl.append(u1T)
        for g in range(GPC):
            u1n_ps = bank16(P, NT * fin)
            for ic in range(NT):
                nc.tensor.transpose(u1n_ps[:, fin * ic:fin * (ic + 1)],
                                    u1Tl[g][0:fin, 128 * ic:128 * (ic + 1)],
                                    eye16[0:fin, 0:fin])
            u1nB = pub.tile([P, NT * fin], BF16, tag="unB", name="u1nB")
            nc.vector.tensor_copy(u1nB[:], u1n_ps)
            u1nBl.append(u1nB)
        for g in range(GPC):
            u2_ps = bank(fin, N)
            for jc in range(NT):
                nc.tensor.matmul(u2_ps, u1nBl[g][:, fin * jc:fin * (jc + 1)],
                                 maskT[g][jc][:], start=(jc == 0),
                                 stop=(jc == NT - 1))
            u2T = pu.tile([fin, N], F16, tag="uT", name="u2T",
                          padded_shape=[128, N])
            nc.scalar.activation(u2T[:], u2_ps, ACTF.Identity)
            u2Tl.append(u2T)
        hT_new = []
        for g in range(GPC):
            oT_ps = bank()
            nc.tensor.matmul(oT_ps, tagw[li][0], hTl[g][:],
                             start=True, stop=False)
            nc.tensor.matmul(oT_ps, tagw[li][1], u1Tl[g][0:fin, :],
                             start=False, stop=False)
            nc.tensor.matmul(oT_ps, tagw[li][2], u2Tl[g][0:fin, :],
                             start=False, stop=True)
            hT = phT.tile([P, N], F16, tag="hT", name="hT")
            nc.scalar.activation(hT[:], oT_ps, ACTF.Lrelu,
                                 bias=tagb[li], alpha=0.01,
                                 accum_out=zpack[g][:, 2 * li:2 * li + 1])
            nc.vector.tensor_reduce(zpack[g][:, 2 * li + 1:2 * li + 2],
                                    hT[:], axis=AXX, op=ALU.max)
            hT_new.append(hT)
        for g in range(GPC):
            hTl[g] = hT_new[g][:]
        if li < 2:
            for g in range(GPC):
                hn_ps = bank16()
                for t in range(NT):
                    nc.tensor.transpose(hn_ps[:, 128 * t:128 * (t + 1)],
                                        hT_new[g][:, 128 * t:128 * (t + 1)],
                                        eye16[:])
                hB = pub.tile([P, N], BF16, tag="unB", name="hnmB")
                nc.vector.tensor_copy(hB[:], hn_ps)
                hnmBl[g] = [hB[:, 128 * t:128 * (t + 1)] for t in range(NT)]

    def ec_layer(f):
        # dynamic kNN-3 on current yT; half-scale Q2 keeps the ranking:
        # Q2'[i,j] = <y_i,y_j> - |y_j|^2/2
        nysql = []
        for g in range(GPC):
            nysq = pwork.tile([P, N], F32, tag="nysq", name="nysq", bufs=2)
            nc.vector.scalar_tensor_tensor(nysq[:], yTl[g][:], -0.5, yTl[g][:],
                                           op0=ALU.mult, op1=ALU.mult)
            nysql.append(nysq)
        Q2l = []
        for g in range(GPC):
            s_ps = sbank()
            nc.tensor.matmul(s_ps, allones[:], nysql[g][:], start=True,
                             stop=True)
            s_sb = pwork.tile([P, N], F32, tag="ssb", name="s_sb", bufs=1)
            nc.scalar.activation(s_sb[:], s_ps, ACTF.Identity)
            Q2 = []
            for t in range(NT):
                gy_ps = bank()
                nc.tensor.matmul(gy_ps, yTl[g][:, 128 * t:128 * (t + 1)],
                                 yTl[g][:], start=True, stop=True)
                q2 = pq.tile([P, N], F32, tag="Q", name="Q2")
                nc.vector.tensor_tensor(q2[:], gy_ps, s_sb[:], op=ALU.add)
                Q2.append(q2)
            Q2l.append(Q2)
        gfnm, afl = [], []
        for g in range(GPC):
            gnm_ps = bank()
            for jc in range(NT):
                nc.tensor.matmul(gnm_ps[:, 128 * jc:128 * (jc + 1)],
                                 yTl[g][:, 128 * jc:128 * (jc + 1)],
                                 ecf_g[f], start=True, stop=True)
            gnm = pnm.tile([P, N], F32, tag="nm", name="gfnm")
            nc.scalar.activation(gnm[:], gnm_ps, ACTF.Identity)
            gfnm.append(gnm)
            af_ps = bank()
            nc.tensor.matmul(af_ps, ecf_a[f], yTl[g][:], start=True, stop=True)
            af = ppay.tile([P, N], F32, tag="pay", name="af")
            nc.scalar.activation(af[:], af_ps, ACTF.Identity, bias=ecf_b[f])
            afl.append(af)
        for g in range(GPC):
            Pl = onehot_rows(marshal_i12(Q2l[g]))
            mx2 = pmx.tile([P, N], F32, tag="mx", name="mx2")
            for l in range(3):
                ga_ps = gather_mm(gfnm[g][:], Pl[l])
                if l == 0:
                    nc.vector.tensor_copy(mx2[:], ga_ps)
                else:
                    nc.vector.tensor_tensor(mx2[:], mx2[:], ga_ps, op=ALU.max)
            nc.vector.tensor_tensor(mx2[:], mx2[:], afl[g][:], op=ALU.add)
            yT_new = pyT.tile([P, N], F32, tag="yT", name="yT2")
            nc.scalar.activation(yT_new[:], mx2[:], ACTF.Lrelu, alpha=0.01,
                                 accum_out=zpack[g][:, 7 + f:8 + f])
            nc.vector.tensor_reduce(zpack[g][:, 10 + f:11 + f], yT_new[:],
                                    axis=AXX, op=ALU.max)
            yTl[g] = yT_new

    tag_layer(0)
    ec_layer(0)
    tag_layer(1)
    ec_layer(1)
    tag_layer(2)

    # ---- pools -> z0 rows ----
    for g in range(GPC):
        means_a = zpack[g][:].rearrange("p (a b) -> p a b", a=6, b=2)[:, 0:3, 0:1]
        nc.vector.tensor_scalar(means_a, means_a, 1.0 / N, 0.0, op0=ALU.mult)
        nc.vector.tensor_scalar(zpack[g][:, 6:9], zpack[g][:, 6:9], 1.0 / N, 0.0,
                                op0=ALU.mult)
        zr_ps = bank(12, P)
        nc.tensor.transpose(zr_ps, zpack[g][:], eye)
        zrow = pwork.tile([12, P], F32, tag="zrow", name="zrow",
                          padded_shape=[128, P], bufs=1)
        nc.scalar.activation(zrow[:], zr_ps, ACTF.Identity)
        dma(io['z0loc'][g].rearrange("(a b) -> a b", a=12), zrow[:])

    # ---- head ----
    cores = list(range(NCORES))
    nc.gpsimd.collective_compute(
        "AllGather", ALU.bypass, replica_groups=[cores],
        ins=[io['z0loc'][:]], outs=[io['z0all'][:]])
    z0s = pst.tile([32, DIM2], F32, tag="z0s", name="z0s", padded_shape=[128, DIM2])
    dma(z0s[:], io['z0all'][:])
    tc.strict_bb_all_engine_barrier()
    zT = []
    spack = pst.tile([P, 12], F32, tag="spack", name="spack")
    qpack = pst.tile([P, 12], F32, tag="qpack", name="qpack")
    junk32 = pst.tile([P, 32], F32, tag="junk32", name="junk32")
    for t in range(12):
        zt_ps = bank(P, 32)
        nc.tensor.transpose(zt_ps, z0s[:, 128 * t:128 * (t + 1)],
                            eye[0:32, 0:32])
        zt = pst.tile([P, 32], F32, tag=f"zT{t}", name=f"zT{t}")
        nc.scalar.activation(zt[:], zt_ps, ACTF.Identity)
        zT.append(zt)
        nc.vector.tensor_reduce(spack[:, t:t + 1], zt[:], axis=AXX, op=ALU.add)
        nc.vector.scalar_tensor_tensor(
            junk32[:], zt[:], 1.0, zt[:], op0=ALU.mult, op1=ALU.mult,
            accum_out=qpack[:, t:t + 1])
    mu = pst.tile([P, 12], F32, tag="mu", name="mu")
    nc.vector.tensor_scalar(mu[:], spack[:], 1.0 / 32, 0.0, op0=ALU.mult)
    m2 = pst.tile([P, 12], F32, tag="m2", name="m2")
    nc.vector.tensor_tensor(m2[:], mu[:], mu[:], op=ALU.mult)
    var = pst.tile([P, 12], F32, tag="var", name="var")
    nc.vector.scalar_tensor_tensor(var[:], qpack[:], 1.0 / 32, m2[:],
                                   op0=ALU.mult, op1=ALU.subtract)
    eps5 = pst.tile([P, 1], F32, tag="eps5", name="eps5")
    nc.any.memset(eps5[:], 1e-5)
    sd = pst.tile([P, 12], F32, tag="sd", name="sd")
    nc.scalar.activation(sd[:], var[:], ACTF.Sqrt, bias=eps5[:])
    inv = pst.tile([P, 12], F32, tag="inv", name="inv")
    nc.vector.reciprocal(inv[:], sd[:])
    gam = wslice('bn_scale')
    bet = wslice('bn_shift')
    sc = pst.tile([P, 12], F32, tag="sc", name="sc")
    nc.vector.tensor_tensor(sc[:], inv[:], gam, op=ALU.mult)
    bi = pst.tile([P, 12], F32, tag="bi", name="bi")
    nc.vector.tensor_tensor(bi[:], mu[:], sc[:], op=ALU.mult)
    nc.vector.tensor_tensor(bi[:], bet, bi[:], op=ALU.subtract)
    zcur = []
    for t in range(12):
        zc = pst.tile([P, 32], F16, tag=f"zc{t}", name=f"zc{t}")
        nc.vector.scalar_tensor_tensor(
            zc[:], zT[t][:], sc[:, t:t + 1], bi[:, t:t + 1].broadcast_to([P, 32]),
            op0=ALU.mult, op1=ALU.add)
        zcur.append(zc[:])

    linB_sb = wslice('linB', 1)
    for li in range(LIN_D):
        if li + 2 < LIN_D:
            load_linw(li + 2)
        wt = wtl[li]
        psL = bank(32, HSH)
        for k in range(12):
            nc.tensor.matmul(psL, zcur[k], wt[:, HSH * k:HSH * (k + 1)],
                             start=(k == 0), stop=False)
        nc.tensor.matmul(psL, ones32[0:1, :],
                         linB_sb[0:1, HSH * li:HSH * (li + 1)],
                         start=False, stop=True)
        zslT = pwork.tile([32, HSH], F32, tag="zslT", name="zslT",
                          padded_shape=[128, HSH], bufs=1)
        nc.scalar.activation(zslT[:], psL, ACTF.Lrelu, alpha=0.01)
        tA_ps = bank(P, 32)
        nc.tensor.transpose(tA_ps, zslT[0:32, 0:128], eye[0:32, 0:32])
        tB_ps = bank(64, 32)
        nc.tensor.transpose(tB_ps, zslT[0:32, 128:HSH], eye[0:32, 0:32])
        zslA = pwork.tile([P, 32], F32, tag="zslA", name="zslA")
        zslB = pwork.tile([64, 32], F32, tag="zslB", name="zslB",
                          padded_shape=[128, 32])
        nc.scalar.activation(zslA[:], tA_ps, ACTF.Identity)
        nc.scalar.activation(zslB[:], tB_ps, ACTF.Identity)
        dma(io['zsl'][li][0:128, :], zslA[:])
        dma(io['zsl'][li][128:HSH, :], zslB[:])
        nc.gpsimd.collective_compute(
            "AllGather", ALU.bypass, replica_groups=[cores],
            ins=[io['zsl'][li][:]], outs=[io['zfull'][li][:]])
        zcat = pwork.tile([P, 12 * 32], F32, tag="zcat", name=f"zcat{li}")
        dma(zcat[:].rearrange("p (k b) -> p k b", k=12),
            io['zfull'][li].rearrange("(k p) b -> p k b", k=12))
        zcat16 = pwork.tile([P, 12 * 32], F16, tag="zcat16", name=f"zcat16_{li}")
        nc.vector.tensor_copy(zcat16[:], zcat[:])
        zcur = [zcat16[:, 32 * t:32 * (t + 1)] for t in range(12)]
    tc.strict_bb_all_engine_barrier()
    outW_sb = wslice('outW')
    outW16 = pst.tile([P, 12], F16, tag="outW16", name="outW16")
    nc.vector.tensor_copy(outW16[:], outW_sb)
    outb_sb = wslice('outb', 1)
    ps_out = bank(1, 32)
    for k in range(12):
        nc.tensor.matmul(ps_out, outW16[:, k:k + 1], zcur[k],
                         start=(k == 0), stop=False)
    nc.tensor.matmul(ps_out, outb_sb, ones32[0:1, :], start=False, stop=True)
    o32 = pwork.tile([1, 32], F32, tag="o32", name="o32", padded_shape=[128, 32])
    nc.scalar.activation(o32[:], ps_out, ACTF.Identity)
    dma(io['out32'][:], o32[:])


def build_nc():
    nc = bacc.Bacc("TRN2", target_bir_lowering=False, debug=False,
                   num_devices=NCORES)
    io = {}

    def inp(name, shape, dtype=F32):
        io[name] = nc.dram_tensor(name, list(shape), dtype,
                                  kind="ExternalInput").ap()

    inp('xaug', (GPC, 8, 2 * N))
    inp('xnmh', (GPC, 128, NT * F), BF16)
    inp('wpack', (128, WPACK_COLS))
    inp('linW', (LIN_D, 12, 128, HSH), F16)

    io['z0loc'] = nc.dram_tensor("z0loc", [GPC, DIM2], F32,
                                 kind="Internal").ap()
    io['out32'] = nc.dram_tensor("out32", [1, 32], F32,
                                 kind="ExternalOutput").ap()
    io['z0all'] = nc.dram_tensor("z0all", [B, DIM2], F32,
                                 addr_space="Shared").ap()
    io['zsl'] = [nc.dram_tensor(f"zsl{li}", [HSH, 32], F32).ap()
                 for li in range(LIN_D)]
    io['zfull'] = [nc.dram_tensor(f"zfull{li}", [DIM2, 32], F32,
                                  addr_space="Shared").ap()
                   for li in range(LIN_D)]

    with tile.TileContext(nc) as tc:
        core_program(tc, io)
    nc.compile()
    return nc


_CACHED = {}


def kernel(**inputs) -> np.ndarray:
    from concourse.bass_utils import run_bass_kernel_spmd
    if 'nc' not in _CACHED:
        _CACHED['nc'] = build_nc()
    nc = _CACHED['nc']
    in_maps = []
    for c in range(NCORES):
        d = prep_host(inputs, c)
        in_maps.append({k: np.ascontiguousarray(v) for k, v in d.items()})
    res = run_bass_kernel_spmd(nc, in_maps, core_ids=list(range(NCORES)),
                               trace=bool(os.environ.get("KBENCH_TRACE")))
    _CACHED['last'] = res
    return res.results[0]['out32'].reshape(-1).astype(np.float32)


if __name__ == "__main__":
    data = dict(np.load('/root/problem/inputs.npz'))
    out = kernel(**data)
    print("kernel out:", out[:5])
